# revision 1
# baseline (speedup 1.0000x reference)
"""GCN 2-layer encoder on 8 Trainium2 NeuronCores (Bass/Tile).

Sharding: nodes in 8 contiguous blocks of 12500 (dst-owner aggregates).
Per layer per core: h = x_local @ W (PE), y = dinv*h published to a
per-quarter AllGather'd table; edge messages fetched with dma_gather
(int16 idx => 4 node-quarters per core-slice, tables <= 25600 rows);
aggregation = one-hot (tensor_scalar is_equal) matmuls accumulating in
PSUM dst-windows of 128 nodes, window-blocks of 12 windows double-
buffered across 6 PSUM banks; self-loop added at window flush.

All CPU-side work is integer graph partitioning/relabeling (sharding);
every float op runs on device, f32 end to end.
"""
import os
import sys

sys.path.insert(0, "/opt/trn_rl_repo")
import numpy as np

import concourse.tile as tile
from concourse import bacc, mybir, library_config
from concourse.bass_utils import run_bass_kernel_spmd

N_NODES = 100000
N_CORES = 8
S = N_NODES // N_CORES          # 12500 nodes per core
D = 128
NW = (S + 127) // 128           # 98 dst windows per core
WPT = 12                        # windows per window-block
NWB = (NW + WPT - 1) // WPT     # 9 window-blocks
QB = [0, 3200, 6400, 9472, S]   # quarter boundaries (tile-aligned starts)
QLEN = [QB[i + 1] - QB[i] for i in range(4)]
CALL = 1024                     # rows per dma_gather (HW-safe max)
f32 = mybir.dt.float32
i16 = mybir.dt.int16


def _quarter_of(m):
    q = np.searchsorted(np.array(QB[1:]), m, side="right")
    return q


def _prep(edge_index):
    """Integer-only graph partitioning -> common SPMD schedule + per-core
    idx/dst arrays. Returns (sched, per_core, deg)."""
    src = np.asarray(edge_index[0], dtype=np.int64)
    dst = np.asarray(edge_index[1], dtype=np.int64)
    deg = np.bincount(dst, minlength=N_NODES).astype(np.int64) + 1

    core = dst // S
    md = dst % S
    w = md // 128                       # window within core
    wb = w // WPT                       # window block
    ms = src % S
    q = _quarter_of(ms)                 # src quarter
    cs = src // S
    idx16 = cs * np.array(QLEN)[q] + (ms - np.array(QB)[q])  # row in table_q

    # per (core, wb, q) segment, edges sorted by (dst, src)
    segs_edges = {}
    order = np.lexsort((src, dst, q, wb, core))
    coreo, wbo, qo = core[order], wb[order], q[order]
    mdo, idxo = md[order], idx16[order]
    key = ((coreo * NWB + wbo) * 4 + qo)
    bounds = np.flatnonzero(np.diff(key)) + 1
    starts = np.concatenate([[0], bounds])
    ends = np.concatenate([bounds, [len(key)]])
    for s0, e0 in zip(starts, ends):
        segs_edges[int(key[s0])] = (mdo[s0:e0], idxo[s0:e0])

    # common segment lengths
    seg_list = [(b, qq) for b in range(NWB) for qq in range(4)]
    L = {}
    for (b, qq) in seg_list:
        mx = 0
        for c in range(N_CORES):
            k = (c * NWB + b) * 4 + qq
            if k in segs_edges:
                mx = max(mx, len(segs_edges[k][0]))
        L[(b, qq)] = max(128, ((mx + 127) // 128) * 128)

    tot_slots = sum(L.values())
    n_chunk_tot = tot_slots // 128

    # per-core padded arrays: gather idx (slot-major) and dstm per slot
    gidx_flat = np.zeros((N_CORES, tot_slots), dtype=np.int16)
    dstm_flat = np.full((N_CORES, tot_slots), -100000.0, dtype=np.float32)
    seg_base = {}
    off = 0
    for (b, qq) in seg_list:
        seg_base[(b, qq)] = off
        for c in range(N_CORES):
            k = (c * NWB + b) * 4 + qq
            if k in segs_edges:
                mdl, idxl = segs_edges[k]
                n = len(mdl)
                gidx_flat[c, off:off + n] = idxl.astype(np.int16)
                dstm_flat[c, off:off + n] = mdl.astype(np.float32)
        off += L[(b, qq)]

    # chunk -> union of touched windows across cores; op list
    # ops[i] = (seg b,q, chunk j, window w, start, stop)
    ops = []
    first_op = {}
    last_op = {}
    for (b, qq) in seg_list:
        base = seg_base[(b, qq)]
        nch = L[(b, qq)] // 128
        for j in range(nch):
            sl = slice(base + 128 * j, base + 128 * (j + 1))
            vals = dstm_flat[:, sl]
            real = vals >= 0
            if not real.any():
                continue
            wins = np.unique((vals[real] // 128).astype(np.int64))
            for wv in wins:
                wv = int(wv)
                oi = len(ops)
                ops.append([b, qq, j, wv, False, False])
                if (b, wv) not in first_op:
                    first_op[(b, wv)] = oi
                last_op[(b, wv)] = oi
    # start/stop must be once per PSUM *bank* per window-block: start=True
    # clears the whole bank (slot-3-only windows survived on HW), so flag
    # only the first/last op among the 4 window slots sharing a bank.
    first_bk, last_bk = {}, {}
    for oi, (b, qq, j, wv, _, _) in enumerate(ops):
        bk = (b, (wv - b * WPT) // 4)
        if bk not in first_bk:
            first_bk[bk] = oi
        last_bk[bk] = oi
    for oi in first_bk.values():
        ops[oi][4] = True
    for oi in last_bk.values():
        ops[oi][5] = True

    # dstm per chunk column (device compare: (iota - dstm) == -128*w)
    dstmT = np.empty((N_CORES, 128, n_chunk_tot), dtype=np.float32)
    for jg in range(n_chunk_tot):
        dstmT[:, :, jg] = dstm_flat[:, 128 * jg:128 * (jg + 1)]

    # wrap gather idx: [128, tot/16], idx i at (i%16, i//16), 8x replicated
    gidx_w = np.empty((N_CORES, 128, tot_slots // 16), dtype=np.int16)
    for c in range(N_CORES):
        a = gidx_flat[c].reshape(-1, 16).T        # [16, tot/16]
        gidx_w[c] = np.tile(a, (8, 1))

    sched = {
        "L": L, "seg_list": seg_list, "seg_base": seg_base,
        "ops": ops, "tot_slots": tot_slots, "n_chunk_tot": n_chunk_tot,
        "windows_per_wb": [min(NW - b * WPT, WPT) for b in range(NWB)],
        "first_op": first_op, "last_op": last_op,
    }
    return sched, gidx_w, dstmT, deg


def _build(sched, repeat=1, debug=False, hoist_collectives=False):
    nc = bacc.Bacc("TRN2", target_bir_lowering=False, debug=False,
                   num_devices=N_CORES)
    NCOL = NW * 128                                   # 12544 padded nodes
    xT = nc.dram_tensor("xT", [128, NCOL], f32, kind="ExternalInput")
    W1 = nc.dram_tensor("W1", [128, 128], f32, kind="ExternalInput")
    W2 = nc.dram_tensor("W2", [128, 128], f32, kind="ExternalInput")
    b1b = nc.dram_tensor("b1b", [128, 128], f32, kind="ExternalInput")
    b2b = nc.dram_tensor("b2b", [128, 128], f32, kind="ExternalInput")
    degi = nc.dram_tensor("degi", [128, NW], f32, kind="ExternalInput")
    iotad = nc.dram_tensor("iotad", [128, 128], f32, kind="ExternalInput")
    identd = nc.dram_tensor("identd", [128, 128], f32, kind="ExternalInput")
    gidx = nc.dram_tensor("gidx", [128, sched["tot_slots"] // 16], i16,
                          kind="ExternalInput")
    dstmT = nc.dram_tensor("dstmT", [128, sched["n_chunk_tot"]], f32,
                           kind="ExternalInput")
    out = nc.dram_tensor("out", [S, D], f32, kind="ExternalOutput")
    if debug:
        dbg_y1 = nc.dram_tensor("dbg_y1", [128, NW * 128], f32,
                                kind="ExternalOutput")
        dbg_b1 = nc.dram_tensor("dbg_b1", [128, NW * 128], f32,
                                kind="ExternalOutput")

    y_slice = nc.dram_tensor("y_slice", [S, D], f32)
    tables = [nc.dram_tensor(f"table{qq}", [N_CORES * QLEN[qq], D], f32,
                             addr_space="Shared") for qq in range(4)]

    seg_list, L, seg_base = sched["seg_list"], sched["L"], sched["seg_base"]
    ops, wpwb = sched["ops"], sched["windows_per_wb"]

    # group ops by segment for emission order
    ops_by_seg = {sk: [] for sk in seg_list}
    for op in ops:
        ops_by_seg[(op[0], op[1])].append(op)

    with tile.TileContext(nc) as tc:
        with (
            tc.tile_pool(name="cst", bufs=1) as cst,
            tc.tile_pool(name="big", bufs=1) as big,
            tc.tile_pool(name="st", bufs=3) as stp,
            tc.tile_pool(name="oh", bufs=4) as ohp,
            tc.tile_pool(name="bank", bufs=1, space="PSUM") as bankp,
            tc.tile_pool(name="php", bufs=2, space="PSUM") as php,
            tc.tile_pool(name="tmp", bufs=3) as tmp,
        ):
            nc.gpsimd.load_library(library_config.mlp)

            xT_sb = cst.tile([128, NCOL], f32, tag="xT")
            W1_sb = cst.tile([128, 128], f32, tag="W1")
            W2_sb = cst.tile([128, 128], f32, tag="W2")
            b1_sb = cst.tile([128, 128], f32, tag="b1")
            b2_sb = cst.tile([128, 128], f32, tag="b2")
            deg_sb = cst.tile([128, NW], f32, tag="deg")
            dinv_sb = cst.tile([128, NW], f32, tag="dinv")
            iota_sb = cst.tile([128, 128], f32, tag="iota")
            id_sb = cst.tile([128, 128], f32, tag="ident")
            gidx_sb = cst.tile([128, sched["tot_slots"] // 16], i16, tag="gx")
            dstm_sb = cst.tile([128, sched["n_chunk_tot"]], f32, tag="dm")
            A = big.tile([128, NCOL], f32, tag="A")   # y_local (self-loop)
            B = big.tile([128, NCOL], f32, tag="B")   # aggregation acc
            banks = [bankp.tile([128, 512], f32, tag=f"bk{i}",
                                name=f"bank{i}")
                     for i in range(6)]

            nc.sync.dma_start(xT_sb[:], xT[:])
            nc.sync.dma_start(W1_sb[:], W1[:])
            nc.sync.dma_start(W2_sb[:], W2[:])
            nc.sync.dma_start(b1_sb[:], b1b[:])
            nc.sync.dma_start(b2_sb[:], b2b[:])
            nc.sync.dma_start(deg_sb[:], degi[:])
            nc.sync.dma_start(iota_sb[:], iotad[:])
            nc.sync.dma_start(id_sb[:], identd[:])
            nc.sync.dma_start(gidx_sb[:], gidx[:])
            nc.sync.dma_start(dstm_sb[:], dstmT[:])
            nc.vector.reciprocal(dinv_sb[:], deg_sb[:])
            nc.scalar.activation(dinv_sb[:], dinv_sb[:],
                                 mybir.ActivationFunctionType.Sqrt)

            def publish_collectives():
                for qq in range(4):
                    nc.gpsimd.collective_compute(
                        "AllGather", mybir.AluOpType.bypass,
                        replica_groups=[list(range(N_CORES))],
                        ins=[y_slice.ap()[QB[qq]:QB[qq + 1], :].opt()],
                        outs=[tables[qq].ap().opt()])

            def publish_quarters(layer):
                """DMA A (y, node-major [p, t*128+f]) quarter slices to
                y_slice rows, then per-quarter AllGather into tables."""
                for qq in range(4):
                    r0, r1 = QB[qq], QB[qq + 1]
                    t0, p0 = r0 // 128, r0 % 128
                    t1, p1 = r1 // 128, r1 % 128
                    assert p0 == 0
                    tf = t1 if p1 else t1
                    if t1 > t0:
                        nc.sync.dma_start(
                            y_slice.ap()[r0:128 * t1, :].rearrange(
                                "(t p) f -> p t f", p=128),
                            A[:, 128 * t0:128 * t1].rearrange(
                                "p (t f) -> p t f", f=128))
                    if p1:
                        nc.sync.dma_start(
                            y_slice.ap()[128 * t1:r1, :],
                            A[0:p1, 128 * t1:128 * (t1 + 1)])
                if not hoist_collectives:
                    publish_collectives()

            def aggregate(layer):
                """Gather + one-hot matmul accumulate + flush into B."""
                for b in range(NWB):
                    nwin = wpwb[b]
                    for qq in range(4):
                        base = seg_base[(b, qq)]
                        Lseg = L[(b, qq)]
                        ncalls = (Lseg + CALL - 1) // CALL
                        stages = []
                        for k in range(ncalls):
                            cl = min(CALL, Lseg - CALL * k)
                            stg = stp.tile([128, CALL // 128, 128], f32,
                                           tag="stg")
                            nc.gpsimd.dma_gather(
                                stg[:, :cl // 128, :], tables[qq].ap(),
                                gidx_sb[:, (base + CALL * k) // 16:
                                        (base + CALL * k + cl) // 16],
                                cl, cl, 128)
                            stages.append(stg)
                        for op in ops_by_seg[(b, qq)]:
                            _, _, j, wv, st_f, sp_f = op
                            jg = (base // 128) + j
                            k, jc = j // (CALL // 128), j % (CALL // 128)
                            S_t = ohp.tile([128, 128], f32, tag="S")
                            nc.vector.tensor_scalar(
                                S_t[:], iota_sb[:],
                                dstm_sb[:, jg:jg + 1], float(-128.0 * wv),
                                op0=mybir.AluOpType.subtract,
                                op1=mybir.AluOpType.is_equal)
                            wl = wv - b * WPT
                            bank = banks[(b % 2) * 3 + wl // 4]
                            bsl = bank[:, 128 * (wl % 4):128 * (wl % 4 + 1)]
                            nc.tensor.matmul(
                                bsl, lhsT=S_t[:], rhs=stages[k][:, jc, :],
                                start=st_f, stop=sp_f)
                    # flush this window block
                    for wl in range(nwin):
                        wv = b * WPT + wl
                        csl = slice(128 * wv, 128 * (wv + 1))
                        bank = banks[(b % 2) * 3 + wl // 4]
                        bsl = bank[:, 128 * (wl % 4):128 * (wl % 4 + 1)]
                        if (b, wv) in sched["first_op"]:
                            nc.vector.tensor_tensor(
                                B[:, csl], bsl, A[:, csl],
                                op=mybir.AluOpType.add)
                        else:
                            nc.vector.tensor_copy(B[:, csl], A[:, csl])

            if hoist_collectives:
                publish_collectives()
                publish_collectives()
            loop_cm = tc.For_i(0, repeat, 1) if repeat > 1 else None
            if loop_cm is not None:
                loop_cm.__enter__()

            # ---------------- layer 1 ----------------
            for t in range(NW):
                h_ps = php.tile([128, 128], f32, tag="php")
                nc.tensor.matmul(h_ps[:], lhsT=xT_sb[:, 128 * t:128 * (t + 1)],
                                 rhs=W1_sb[:], start=True, stop=True)
                nc.vector.tensor_scalar(A[:, 128 * t:128 * (t + 1)], h_ps[:],
                                        dinv_sb[:, t:t + 1], None,
                                        op0=mybir.AluOpType.mult)
            if debug:
                nc.sync.dma_start(dbg_y1.ap(), A[:])
            publish_quarters(0)
            aggregate(0)
            if debug:
                nc.sync.dma_start(dbg_b1.ap(), B[:])
            # out1 = B*dinv + b1 ; z = relu(out1) -> A
            for t in range(NW):
                csl = slice(128 * t, 128 * (t + 1))
                nc.vector.tensor_scalar(B[:, csl], B[:, csl],
                                        dinv_sb[:, t:t + 1], None,
                                        op0=mybir.AluOpType.mult)
                nc.vector.tensor_tensor(B[:, csl], B[:, csl], b1_sb[:],
                                        op=mybir.AluOpType.add)
            nc.scalar.activation(A[:], B[:],
                                 mybir.ActivationFunctionType.Relu)

            # ---------------- layer 2 ----------------
            for t in range(NW):
                csl = slice(128 * t, 128 * (t + 1))
                t_ps = php.tile([128, 128], f32, tag="php")
                nc.tensor.transpose(t_ps[:], A[:, csl], id_sb[:])
                zT = tmp.tile([128, 128], f32, tag="zT")
                nc.vector.tensor_copy(zT[:], t_ps[:])
                h_ps = php.tile([128, 128], f32, tag="php")
                nc.tensor.matmul(h_ps[:], lhsT=zT[:], rhs=W2_sb[:],
                                 start=True, stop=True)
                nc.vector.tensor_scalar(A[:, csl], h_ps[:],
                                        dinv_sb[:, t:t + 1], None,
                                        op0=mybir.AluOpType.mult)
            publish_quarters(1)
            aggregate(1)
            for t in range(NW):
                csl = slice(128 * t, 128 * (t + 1))
                nc.vector.tensor_scalar(B[:, csl], B[:, csl],
                                        dinv_sb[:, t:t + 1], None,
                                        op0=mybir.AluOpType.mult)
                nc.vector.tensor_tensor(B[:, csl], B[:, csl], b2_sb[:],
                                        op=mybir.AluOpType.add)
            # write output rows 0..12500
            nc.sync.dma_start(
                out.ap()[0:128 * 97, :].rearrange("(t p) f -> p t f", p=128),
                B[:, 0:128 * 97].rearrange("p (t f) -> p t f", f=128))
            nc.sync.dma_start(out.ap()[128 * 97:S, :],
                              B[0:S - 128 * 97, 128 * 97:128 * 98])

            if loop_cm is not None:
                loop_cm.__exit__(None, None, None)

    nc.compile()
    return nc


def _make_in_maps(x, W1, b1, W2, b2, sched, gidx_w, dstmT, deg):
    NCOL = NW * 128
    iota = np.broadcast_to(np.arange(128, dtype=np.float32),
                           (128, 128)).copy()
    ident = np.eye(128, dtype=np.float32)
    in_maps = []
    for c in range(N_CORES):
        xs = x[S * c:S * (c + 1)].astype(np.float32)
        xT = np.zeros((128, NCOL), np.float32)
        xT[:, :S] = xs.T
        # reorder columns to node-major [p=n%128, t=n//128] layout:
        # xT column layout IS n along free dim; device expects col=t*128+?:
        # lhsT tile t = xT[:, 128t:128(t+1)] = features x rows ✓ already.
        degc = deg[S * c:S * (c + 1)].astype(np.float32)
        degp = np.ones(NCOL, np.float32)
        degp[:S] = degc
        # node n -> [n%128, n//128]
        deg_pc = degp.reshape(NW, 128).T.copy()
        in_maps.append({
            "xT": xT,
            "W1": W1.astype(np.float32), "W2": W2.astype(np.float32),
            "b1b": np.broadcast_to(b1.astype(np.float32), (128, 128)).copy(),
            "b2b": np.broadcast_to(b2.astype(np.float32), (128, 128)).copy(),
            "degi": deg_pc, "iotad": iota, "identd": ident,
            "gidx": gidx_w[c], "dstmT": dstmT[c],
        })
    return in_maps


def kernel(x, edge_index, W1, b1, W2, b2):
    sched, gidx_w, dstmT, deg = _prep(edge_index)
    nc = _build(sched, repeat=int(os.environ.get("KERNEL_REPEAT", "1")))
    in_maps = _make_in_maps(x, W1, b1, W2, b2, sched, gidx_w, dstmT, deg)
    res = run_bass_kernel_spmd(nc, in_maps, core_ids=list(range(N_CORES)))
    return np.concatenate([res.results[c]["out"] for c in range(N_CORES)], 0)



# revision 2
# speedup vs baseline: 19.5829x; 19.5829x over previous
"""GCN 2-layer encoder on 8 Trainium2 NeuronCores (Bass/Tile) - v2.

Sharding: nodes in 8 contiguous blocks of 12500 (dst-owner aggregates).
v2 vs v1:
  * fp16 datapath (tables, gathers, one-hots, matmuls); f32 PSUM accum.
  * transposed aggregation: bank_T[feat, dst] += stage[slot, feat].T @
    S[slot, dstspan]; boundary chunks use one wider matmul instead of
    two ops; self-loop + bias folded into PE (identity matmul per
    window); per-bank (not per-window) flush with fused bias+relu.
  * multi-queue SWDGE gathers (desc generation was the v1 bottleneck:
    ~5.2 ns/descriptor single-queue).
  * host precomputes dinv and per-op shifted dst-position columns (f16
    exact: in-span values < 2048, sentinel 8192).
"""
import os
import sys

sys.path.insert(0, "/opt/trn_rl_repo")
import numpy as np

import concourse.tile as tile
from concourse import bacc, mybir, library_config
from concourse.bass_utils import run_bass_kernel_spmd

N_NODES = 100000
N_CORES = 8
S = N_NODES // N_CORES          # 12500 nodes per core
D = 128
NW = (S + 127) // 128           # 98 dst windows per core
WPT = 8                         # windows per window-block
NWB = (NW + WPT - 1) // WPT     # 9 window-blocks
QB = [0, 3200, 6400, 9472, S]   # quarter boundaries (128-aligned starts)
QLEN = [QB[i + 1] - QB[i] for i in range(4)]
CALL = 1024                     # rows per dma_gather (HW-safe max)
NQ = int(os.environ.get("KERNEL_NQ", "3"))  # SWDGE queues
f32 = mybir.dt.float32
f16 = mybir.dt.float16
i16 = mybir.dt.int16
SENT = 8192.0                   # f16-exact sentinel, never matches iota


def _quarter_of(m):
    return np.searchsorted(np.array(QB[1:]), m, side="right")


def _prep(edge_index):
    """Integer graph partitioning -> common SPMD schedule + per-core
    gather-index / dst-position arrays."""
    src = np.asarray(edge_index[0], dtype=np.int64)
    dst = np.asarray(edge_index[1], dtype=np.int64)
    deg = np.bincount(dst, minlength=N_NODES).astype(np.int64) + 1

    core = dst // S
    md = dst % S
    w = md // 128                       # window within core
    wb = w // WPT                       # window block
    ms = src % S
    q = _quarter_of(ms)                 # src quarter
    cs = src // S
    idx16 = cs * np.array(QLEN)[q] + (ms - np.array(QB)[q])  # row in table_q

    segs_edges = {}
    order = np.lexsort((src, dst, q, wb, core))
    coreo, wbo, qo = core[order], wb[order], q[order]
    mdo, idxo = md[order], idx16[order]
    key = ((coreo * NWB + wbo) * 4 + qo)
    bounds = np.flatnonzero(np.diff(key)) + 1
    starts = np.concatenate([[0], bounds])
    ends = np.concatenate([bounds, [len(key)]])
    for s0, e0 in zip(starts, ends):
        segs_edges[int(key[s0])] = (mdo[s0:e0], idxo[s0:e0])

    seg_list = [(b, qq) for b in range(NWB) for qq in range(4)]
    L = {}
    for (b, qq) in seg_list:
        mx = 0
        for c in range(N_CORES):
            k = (c * NWB + b) * 4 + qq
            if k in segs_edges:
                mx = max(mx, len(segs_edges[k][0]))
        L[(b, qq)] = max(128, ((mx + 127) // 128) * 128)

    tot_slots = sum(L.values())
    n_chunk_tot = tot_slots // 128

    gidx_flat = np.zeros((N_CORES, tot_slots), dtype=np.int16)
    dstm_flat = np.full((N_CORES, tot_slots), -1.0, dtype=np.float64)
    seg_base = {}
    off = 0
    for (b, qq) in seg_list:
        seg_base[(b, qq)] = off
        for c in range(N_CORES):
            k = (c * NWB + b) * 4 + qq
            if k in segs_edges:
                mdl, idxl = segs_edges[k]
                n = len(mdl)
                gidx_flat[c, off:off + n] = idxl.astype(np.int16)
                dstm_flat[c, off:off + n] = mdl
        off += L[(b, qq)]

    # ops: per (segment-chunk, psum-bank) with union window span across
    # cores. op = [b, qq, j(chunk in seg), kbank(local), wlo, whi]
    ops = []
    for (b, qq) in seg_list:
        base = seg_base[(b, qq)]
        nch = L[(b, qq)] // 128
        for j in range(nch):
            sl = slice(base + 128 * j, base + 128 * (j + 1))
            vals = dstm_flat[:, sl]
            real = vals >= 0
            if not real.any():
                continue
            wins = (vals[real].astype(np.int64) // 128)
            wl_min = int(wins.min()) - b * WPT
            wl_max = int(wins.max()) - b * WPT
            assert 0 <= wl_min <= wl_max < WPT
            for kb in range(wl_min // 4, wl_max // 4 + 1):
                lo = max(wl_min, 4 * kb)
                hi = min(wl_max, 4 * kb + 3)
                ops.append([b, qq, j, kb, lo, hi])

    # per-op per-core f16 dst-position columns, shifted to op window base
    n_ops = len(ops)
    dstm_ops = np.full((N_CORES, 128, n_ops), SENT, dtype=np.float32)
    for oi, (b, qq, j, kb, lo, hi) in enumerate(ops):
        base = seg_base[(b, qq)]
        sl = slice(base + 128 * j, base + 128 * (j + 1))
        vals = dstm_flat[:, sl]                     # [8, 128]
        shift = 128.0 * (b * WPT + lo)
        span = (hi - lo + 1) * 128
        enc = vals - shift
        ok = (vals >= 0) & (enc >= 0) & (enc < span)
        encf = np.where(ok, enc, SENT).astype(np.float32)
        dstm_ops[:, :, oi] = encf

    # wrap gather idx: [128, tot/16], idx i at (i%16, i//16), 8x replicated
    gidx_w = np.empty((N_CORES, 128, tot_slots // 16), dtype=np.int16)
    for c in range(N_CORES):
        a = gidx_flat[c].reshape(-1, 16).T
        gidx_w[c] = np.tile(a, (8, 1))

    sched = {
        "L": L, "seg_list": seg_list, "seg_base": seg_base,
        "ops": ops, "tot_slots": tot_slots, "n_chunk_tot": n_chunk_tot,
        "n_ops": n_ops,
        "windows_per_wb": [min(NW - b * WPT, WPT) for b in range(NWB)],
    }
    return sched, gidx_w, dstm_ops, deg


def _build(sched, repeat=1, hoist_collectives=False):
    nc = bacc.Bacc("TRN2", target_bir_lowering=False, debug=False,
                   num_devices=N_CORES, num_swdge_queues=NQ)
    NCOL = NW * 128                                   # 12544 padded nodes
    xT = nc.dram_tensor("xT", [128, NCOL], f16, kind="ExternalInput")
    W1 = nc.dram_tensor("W1", [128, 128], f16, kind="ExternalInput")
    W2 = nc.dram_tensor("W2", [128, 128], f16, kind="ExternalInput")
    b1c = nc.dram_tensor("b1c", [128, 1], f32, kind="ExternalInput")
    b2c = nc.dram_tensor("b2c", [128, 1], f32, kind="ExternalInput")
    dinvc = nc.dram_tensor("dinvc", [128, NW], f32, kind="ExternalInput")
    dinvb = nc.dram_tensor("dinvb", [128, NCOL], f16, kind="ExternalInput")
    iotad = nc.dram_tensor("iotad", [128, 512], f16, kind="ExternalInput")
    identd = nc.dram_tensor("identd", [128, 128], f16, kind="ExternalInput")
    gidx = nc.dram_tensor("gidx", [128, sched["tot_slots"] // 16], i16,
                          kind="ExternalInput")
    dstmd = nc.dram_tensor("dstmd", [128, sched["n_ops"]], f32,
                           kind="ExternalInput")
    out = nc.dram_tensor("out", [S, D], f16, kind="ExternalOutput")

    y_slice = nc.dram_tensor("y_slice", [S, D], f16)
    tables = [nc.dram_tensor(f"table{qq}", [N_CORES * QLEN[qq], D], f16,
                             addr_space="Shared") for qq in range(4)]

    seg_list, L, seg_base = sched["seg_list"], sched["L"], sched["seg_base"]
    ops, wpwb = sched["ops"], sched["windows_per_wb"]

    # group ops by (block, bank) for emission + start/stop flags
    ops_by_seg = {}
    for oi, op in enumerate(ops):
        ops_by_seg.setdefault((op[0], op[1]), []).append((oi, op))

    with tile.TileContext(nc) as tc:
        with (
            tc.tile_pool(name="cst", bufs=1) as cst,
            tc.tile_pool(name="big", bufs=1) as big,
            tc.tile_pool(name="st", bufs=6) as stp,
            tc.tile_pool(name="oh", bufs=6) as ohp,
            tc.tile_pool(name="bank", bufs=1, space="PSUM") as bankp,
            tc.tile_pool(name="php", bufs=2, space="PSUM") as php,
            tc.tile_pool(name="tp", bufs=2, space="PSUM") as tpp,
        ):
            nc.gpsimd.load_library(library_config.mlp)

            xT_sb = cst.tile([128, NCOL], f16, tag="xT")
            W1_sb = cst.tile([128, 128], f16, tag="W1")
            W2_sb = cst.tile([128, 128], f16, tag="W2")
            b1_sb = cst.tile([128, 1], f32, tag="b1")
            b2_sb = cst.tile([128, 1], f32, tag="b2")
            dinvc_sb = cst.tile([128, NW], f32, tag="dinvc")
            dinvb_sb = cst.tile([128, NCOL], f16, tag="dinvb")
            iota_sb = cst.tile([128, 512], f16, tag="iota")
            id_sb = cst.tile([128, 128], f16, tag="ident")
            gidx_sb = cst.tile([128, sched["tot_slots"] // 16], i16, tag="gx")
            dstm_sb = cst.tile([128, sched["n_ops"]], f32, tag="dm")
            A = big.tile([128, NCOL], f16, tag="A")    # y node-major
            BT = big.tile([128, NCOL], f16, tag="BT")  # out_T / z_T
            OT = big.tile([128, NCOL], f16, tag="OT")  # layer2 out_T
            OS = big.tile([128, NCOL], f16, tag="OS")  # final node-major
            banks = [bankp.tile([128, 512], f32, tag=f"bk{i}",
                                name=f"bank{i}")
                     for i in range(4)]

            nc.sync.dma_start(xT_sb[:], xT[:])
            nc.sync.dma_start(W1_sb[:], W1[:])
            nc.sync.dma_start(W2_sb[:], W2[:])
            nc.sync.dma_start(b1_sb[:], b1c[:])
            nc.sync.dma_start(b2_sb[:], b2c[:])
            nc.sync.dma_start(dinvc_sb[:], dinvc[:])
            nc.sync.dma_start(dinvb_sb[:], dinvb[:])
            nc.sync.dma_start(iota_sb[:], iotad[:])
            nc.sync.dma_start(id_sb[:], identd[:])
            nc.sync.dma_start(gidx_sb[:], gidx[:])
            nc.sync.dma_start(dstm_sb[:], dstmd[:])

            def publish_collectives():
                for qq in range(4):
                    nc.gpsimd.collective_compute(
                        "AllGather", mybir.AluOpType.bypass,
                        replica_groups=[list(range(N_CORES))],
                        ins=[y_slice.ap()[QB[qq]:QB[qq + 1], :].opt()],
                        outs=[tables[qq].ap().opt()])

            def publish_quarters():
                """DMA A (y, node-major [p, t*128+f]) quarter slices to
                y_slice rows, then per-quarter AllGather into tables."""
                for qq in range(4):
                    r0, r1 = QB[qq], QB[qq + 1]
                    t0 = r0 // 128
                    t1, p1 = r1 // 128, r1 % 128
                    if t1 > t0:
                        nc.sync.dma_start(
                            y_slice.ap()[r0:128 * t1, :].rearrange(
                                "(t p) f -> p t f", p=128),
                            A[:, 128 * t0:128 * t1].rearrange(
                                "p (t f) -> p t f", f=128))
                    if p1:
                        nc.sync.dma_start(
                            y_slice.ap()[128 * t1:r1, :],
                            A[0:p1, 128 * t1:128 * (t1 + 1)])
                if not hoist_collectives:
                    publish_collectives()

            gq = [0]  # gather queue round-robin counter

            MAXB = int(os.environ.get("KERNEL_MAXBLOCKS", str(NWB)))

            def aggregate(out_T, bias_sb, relu):
                """Gather + transposed one-hot matmul accumulate; flush
                into out_T with dinv scale, bias add, optional relu."""
                for b in range(min(NWB, MAXB)):
                    nwin = wpwb[b]
                    nbank = (nwin + 3) // 4
                    # last edge op (global oi) per bank, for stop flags
                    bank_last = [None] * nbank
                    n_bank_ops = [0] * nbank
                    for qq in range(4):
                        for oi, op in ops_by_seg.get((b, qq), []):
                            bank_last[op[3]] = oi
                            n_bank_ops[op[3]] += 1
                    # self-loops first: bank_T[:, wl] += A_win.T
                    for kb in range(nbank):
                        bank = banks[(b % 2) * 2 + kb]
                        for wl in range(4 * kb, min(4 * kb + 4, nwin)):
                            wv = b * WPT + wl
                            csl = slice(128 * wv, 128 * (wv + 1))
                            st_f = (wl == 4 * kb)
                            sp_f = (wl == min(4 * kb + 4, nwin) - 1
                                    and bank_last[kb] is None)
                            nc.tensor.matmul(
                                bank[:, 128 * (wl % 4):128 * (wl % 4 + 1)],
                                lhsT=A[:, csl], rhs=id_sb[:],
                                start=st_f, stop=sp_f)
                    # per quarter: gather, then all its ops (both banks)
                    for qq in range(4):
                        base = seg_base[(b, qq)]
                        Lseg = L[(b, qq)]
                        ncalls = (Lseg + CALL - 1) // CALL
                        stages = []
                        for k in range(ncalls):
                            cl = min(CALL, Lseg - CALL * k)
                            stg = stp.tile([128, CALL // 128, 128], f16,
                                           tag="stg")
                            nc.gpsimd.dma_gather(
                                stg[:, :cl // 128, :], tables[qq].ap(),
                                gidx_sb[:, (base + CALL * k) // 16:
                                        (base + CALL * k + cl) // 16],
                                cl, cl, 128, queue_num=gq[0] % NQ)
                            gq[0] += 1
                            stages.append(stg)
                        for oi, op in ops_by_seg.get((b, qq), []):
                            _, _, j, kb, lo, hi = op
                            bank = banks[(b % 2) * 2 + kb]
                            span = (hi - lo + 1) * 128
                            k, jc = j // (CALL // 128), j % (CALL // 128)
                            S_t = ohp.tile([128, 512], f16, tag="S")
                            nc.vector.tensor_scalar(
                                S_t[:, :span], iota_sb[:, :span],
                                dstm_sb[:, oi:oi + 1], 0.0,
                                op0=mybir.AluOpType.subtract,
                                op1=mybir.AluOpType.is_equal)
                            c0 = 128 * (lo - 4 * kb)
                            nc.tensor.matmul(
                                bank[:, c0:c0 + span],
                                lhsT=stages[k][:, jc, :],
                                rhs=S_t[:, :span],
                                start=False,
                                stop=(oi == bank_last[kb]))
                    # flush banks -> out_T columns, fused dinv/bias/relu
                    for kb in range(nbank):
                        bank = banks[(b % 2) * 2 + kb]
                        ncols = 128 * (min(4 * kb + 4, nwin) - 4 * kb)
                        c0 = 128 * (b * WPT + 4 * kb)
                        nc.vector.tensor_tensor(
                            out_T[:, c0:c0 + ncols], bank[:, :ncols],
                            dinvb_sb[:, c0:c0 + ncols],
                            op=mybir.AluOpType.mult)
                        if relu:
                            nc.vector.tensor_scalar(
                                out_T[:, c0:c0 + ncols],
                                out_T[:, c0:c0 + ncols],
                                bias_sb[:, 0:1], 0.0,
                                op0=mybir.AluOpType.add,
                                op1=mybir.AluOpType.max)
                        else:
                            nc.vector.tensor_scalar(
                                out_T[:, c0:c0 + ncols],
                                out_T[:, c0:c0 + ncols],
                                bias_sb[:, 0:1], None,
                                op0=mybir.AluOpType.add)

            if hoist_collectives and not os.environ.get("KERNEL_NO_AG"):
                publish_collectives()
                publish_collectives()
            loop_cm = tc.For_i(0, repeat, 1) if repeat > 1 else None
            if loop_cm is not None:
                loop_cm.__enter__()

            # ---------------- layer 1 ----------------
            for t in range(NW):
                h_ps = php.tile([128, 128], f32, tag="php")
                nc.tensor.matmul(h_ps[:], lhsT=xT_sb[:, 128 * t:128 * (t + 1)],
                                 rhs=W1_sb[:], start=True, stop=True)
                nc.vector.tensor_scalar(A[:, 128 * t:128 * (t + 1)], h_ps[:],
                                        dinvc_sb[:, t:t + 1], None,
                                        op0=mybir.AluOpType.mult)
            publish_quarters()
            if not os.environ.get("KERNEL_SKIP_AGG"):
                aggregate(BT, b1_sb, relu=True)  # BT = z_T = relu(out1_T)

            # ---------------- layer 2 ----------------
            for t in range(NW):
                csl = slice(128 * t, 128 * (t + 1))
                h_ps = php.tile([128, 128], f32, tag="php")
                nc.tensor.matmul(h_ps[:], lhsT=BT[:, csl], rhs=W2_sb[:],
                                 start=True, stop=True)
                nc.vector.tensor_scalar(A[:, csl], h_ps[:],
                                        dinvc_sb[:, t:t + 1], None,
                                        op0=mybir.AluOpType.mult)
            publish_quarters()
            if not os.environ.get("KERNEL_SKIP_AGG"):
                aggregate(OT, b2_sb, relu=False)  # OT = out2_T

            # transpose back to node-major and write out
            for t in range(NW):
                csl = slice(128 * t, 128 * (t + 1))
                t_ps = tpp.tile([128, 128], f16, tag="phpt")
                nc.tensor.transpose(t_ps[:], OT[:, csl], id_sb[:])
                nc.scalar.copy(OS[:, csl], t_ps[:])
            nc.sync.dma_start(
                out.ap()[0:128 * 97, :].rearrange("(t p) f -> p t f", p=128),
                OS[:, 0:128 * 97].rearrange("p (t f) -> p t f", f=128))
            nc.sync.dma_start(out.ap()[128 * 97:S, :],
                              OS[0:S - 128 * 97, 128 * 97:128 * 98])

            if loop_cm is not None:
                loop_cm.__exit__(None, None, None)

    nc.compile()
    return nc


def _make_in_maps(x, W1, b1, W2, b2, sched, gidx_w, dstm_ops, deg):
    NCOL = NW * 128
    iota = np.broadcast_to(np.arange(512, dtype=np.float16),
                           (128, 512)).copy()
    ident = np.eye(128, dtype=np.float16)
    dinv = 1.0 / np.sqrt(deg.astype(np.float64))
    in_maps = []
    for c in range(N_CORES):
        xs = x[S * c:S * (c + 1)].astype(np.float16)
        xT = np.zeros((128, NCOL), np.float16)
        xT[:, :S] = xs.T
        dinvc_full = np.ones(NCOL, np.float64)
        dinvc_full[:S] = dinv[S * c:S * (c + 1)]
        dinv_pc = dinvc_full.reshape(NW, 128).T.astype(np.float32).copy()
        dinv_b = np.broadcast_to(dinvc_full.astype(np.float16),
                                 (128, NCOL)).copy()
        in_maps.append({
            "xT": xT,
            "W1": W1.astype(np.float16), "W2": W2.astype(np.float16),
            "b1c": np.asarray(b1, np.float32).reshape(128, 1).copy(),
            "b2c": np.asarray(b2, np.float32).reshape(128, 1).copy(),
            "dinvc": dinv_pc, "dinvb": dinv_b,
            "iotad": iota, "identd": ident,
            "gidx": gidx_w[c], "dstmd": dstm_ops[c],
        })
    return in_maps


def kernel(x, edge_index, W1, b1, W2, b2):
    sched, gidx_w, dstm_ops, deg = _prep(edge_index)
    nc = _build(sched, repeat=int(os.environ.get("KERNEL_REPEAT", "1")))
    in_maps = _make_in_maps(x, W1, b1, W2, b2, sched, gidx_w, dstm_ops, deg)
    res = run_bass_kernel_spmd(nc, in_maps, core_ids=list(range(N_CORES)))
    return np.concatenate(
        [res.results[c]["out"].astype(np.float32) for c in range(N_CORES)], 0)
